# revision 19
# baseline (speedup 1.0000x reference)
"""Trainium2 Bass kernel for a MANN-style memory step (scatter_memory).

Contract: kernel(**inputs) takes FULL unsharded inputs, returns the full
outputs (out, new_M, new_w_u, new_prev_w_r) like the reference.

Strategy: data-parallel over batch across 8 NeuronCores. W, b, M are
replicated; X, w_u, prev_w_r are sharded on batch. The einsum
k_w = w_w^T @ h is computed per-core and combined with a ReduceScatter;
each core finalizes + writes its 256-row shard of new_M.
"""

import os

import numpy as np

import concourse.bass as bass
import concourse.mybir as mybir
import concourse.tile as tile
from concourse import bacc
from concourse.bass import ts
from concourse.bass_utils import run_bass_kernel_spmd
from concourse.masks import make_identity

F32 = mybir.dt.float32
BF16 = mybir.dt.bfloat16
AX = mybir.AxisListType.X
OP = mybir.AluOpType
AF = mybir.ActivationFunctionType

B, IN, H, MEM = 16384, 512, 512, 2048
NCORES = 8
BL = B // NCORES            # 2048 rows per core
NBT = BL // 128             # 16 batch tiles per core
GRP = 4                     # b-tiles per k_w accumulation group
NGRP = NBT // GRP
KC = IN // 128              # 4 contraction chunks for X@W
MC = MEM // 128             # 16 memory chunks
NC4 = MEM // 512            # 4 n-chunks of 512 for inner matmul
MSH = MEM // NCORES         # 256 memory rows per core shard

GATE = float(1.0 / (1.0 + np.exp(0.4)))
ONE_MINUS_GATE = float(1.0 - GATE)
GAMMA = 0.3
LEAK = 0.01


def _build_body(nc, tc, io, nbt=NBT, stage=5):
    X_d = io["X_l"]
    W_d = io["W"]
    b_d = io["b"]
    M_d = io["M"]
    Msh_d = io["M_shard"]
    wu_d = io["w_u_l"]
    pw_d = io["prev_w_r_l"]
    out_d = io["out_l"]
    newM_d = io["newM_l"]
    nwu_d = io["new_w_u_l"]
    nwr_d = io["new_prev_w_r_l"]

    ctx = io["_ctx"]
    const = ctx.enter_context(tc.tile_pool(name="const", bufs=1))
    dpool = ctx.enter_context(tc.tile_pool(name="dpool", bufs=1))
    xpool = ctx.enter_context(tc.tile_pool(name="xpool", bufs=2))
    xTpool = ctx.enter_context(tc.tile_pool(name="xTpool", bufs=2))
    hpool = ctx.enter_context(tc.tile_pool(name="hpool", bufs=GRP + 1))
    hTpool = ctx.enter_context(tc.tile_pool(name="hTpool", bufs=2))
    wupool = ctx.enter_context(tc.tile_pool(name="wupool", bufs=2))
    pwpool = ctx.enter_context(tc.tile_pool(name="pwpool", bufs=GRP))
    epool = ctx.enter_context(tc.tile_pool(name="epool", bufs=2))
    ohpool = ctx.enter_context(tc.tile_pool(name="ohpool", bufs=2))
    wrTpool = ctx.enter_context(tc.tile_pool(name="wrTpool", bufs=2))
    rdpool = ctx.enter_context(tc.tile_pool(name="rdpool", bufs=2))
    scrpool = ctx.enter_context(tc.tile_pool(name="scrpool", bufs=1))
    stat = ctx.enter_context(tc.tile_pool(name="stat", bufs=2))
    tail = ctx.enter_context(tc.tile_pool(name="tail", bufs=1))
    dram = ctx.enter_context(tc.tile_pool(name="dram", bufs=1, space="DRAM"))

    ptr = ctx.enter_context(tc.tile_pool(name="ptr", bufs=2, space="PSUM"))
    ph = ctx.enter_context(tc.tile_pool(name="ph", bufs=1, space="PSUM"))
    pin = ctx.enter_context(tc.tile_pool(name="pin", bufs=2, space="PSUM"))
    prd = ctx.enter_context(tc.tile_pool(name="prd", bufs=2, space="PSUM"))
    pkw = ctx.enter_context(tc.tile_pool(name="pkw", bufs=1, space="PSUM"))

    # ---------------- setup: constants ----------------
    sub = os.environ.get("SETUP", "d")
    ident = const.tile([128, 128], F32)
    make_identity(nc, ident[:])
    if sub == "a":
        return

    W_sb = const.tile([128, KC, H], F32)
    nc.sync.dma_start(W_sb[:], W_d[:].rearrange("(c p) h -> p c h", p=128))

    bias_sb = const.tile([1, H], F32)
    nc.sync.dma_start(bias_sb[:], b_d[:])
    ones1 = const.tile([1, 128], F32)
    nc.vector.memset(ones1[:], 1.0)

    M_sb = const.tile([128, MC, H], F32)
    nc.sync.dma_start(M_sb[:], M_d[:].rearrange("(m p) h -> p m h", p=128))

    if sub == "b":
        return
    # row norms of M -> rmn = 1/||M_m|| per memory slot
    mn = const.tile([128, MC], F32)
    rmn = const.tile([128, MC], F32)
    for m in range(MC):
        scr = scrpool.tile([128, H], F32, name="scr_setup", tag="scr")
        nc.scalar.activation(
            scr[:], M_sb[:, m, :], AF.Square, accum_out=mn[:, m : m + 1]
        )
    mn_s = const.tile([128, MC], F32)
    nc.scalar.activation(mn_s[:], mn[:], AF.Sqrt)
    nc.vector.reciprocal(rmn[:], mn_s[:])

    if sub == "c":
        return
    # M_unit^T (bf16): chunk hc holds rows [128] of H, all MEM columns.
    # Built via out = M_chunk.T @ diag(rmn_chunk).
    MuT = const.tile([128, KC, MEM], BF16)
    for hc in range(KC):
        for mq in range(MC // 4):
            ps = ptr.tile([128, 512], F32, name="ps_mut", tag="tr")
            for j in range(4):
                m = 4 * mq + j
                d_t = dpool.tile([128, 128], F32, name="d_t")
                nc.vector.tensor_scalar(
                    out=d_t[:], in0=ident[:], scalar1=rmn[:, m : m + 1],
                    scalar2=None, op0=OP.mult,
                )
                nc.tensor.matmul(
                    ps[:, ts(j, 128)], M_sb[:, m, ts(hc, 128)], d_t[:],
                    start=True, stop=True,
                )
            nc.vector.tensor_copy(MuT[:, hc, ts(mq, 512)], ps[:])

    kw_acc = const.tile([128, MC, H], F32)

    if stage < 1:
        return
    use_bias_mm = os.environ.get("KBIAS", "1") == "1"

    # ---------------- main loop over batch tiles ----------------
    for g in range(nbt // GRP):
        h_tiles = []
        ww_tiles = []
        for t in range(GRP):
            bt = g * GRP + t

            # ---- controller: h = leaky_relu(X @ W + b) ----
            x_t = xpool.tile([128, IN], F32, name="x_t")
            nc.sync.dma_start(x_t[:], X_d[ts(bt, 128), :])
            psx = ptr.tile([128, IN], F32, name="psx", tag="tr")
            for c in range(KC):
                nc.tensor.transpose(psx[:, ts(c, 128)], x_t[:, ts(c, 128)], ident[:])
            xT = xTpool.tile([128, KC, 128], F32, name="xT")
            nc.vector.tensor_copy(xT[:], psx[:])

            psh = ph.tile([128, H], F32, name="psh")
            for c in range(KC):
                nc.tensor.matmul(
                    psh[:], xT[:, c, :], W_sb[:, c, :],
                    start=(c == 0), stop=(not use_bias_mm and c == KC - 1),
                )
            if use_bias_mm:
                nc.tensor.matmul(
                    psh[:], ones1[:], bias_sb[:], start=False, stop=True,
                )
            h_t = hpool.tile([128, H], F32, name="h_t")
            lmask = scrpool.tile([128, H], F32, name="lmask", tag="scr")
            nc.vector.tensor_scalar(
                out=lmask[:], in0=psh[:], scalar1=0.0,
                scalar2=1.0 - LEAK, op0=OP.is_gt, op1=OP.mult,
            )
            nc.vector.scalar_tensor_tensor(
                out=h_t[:], in0=lmask[:], scalar=LEAK, in1=psh[:],
                op0=OP.add, op1=OP.mult,
            )
            h_tiles.append(h_t)
            nc.sync.dma_start(out_d[ts(bt, 128), 0:H], h_t[:])

            if stage < 2:
                continue
            # ---- 1/||h|| ----
            hsq = stat.tile([128, 1], F32, name="hsq")
            scrh = scrpool.tile([128, H], F32, name="scrh", tag="scr")
            nc.scalar.activation(
                scrh[:], h_t[:], AF.Square, accum_out=hsq[:]
            )
            hn_s = stat.tile([128, 1], F32, name="hn_s")
            nc.scalar.activation(hn_s[:], hsq[:], AF.Sqrt)
            rhn = stat.tile([128, 1], F32, name="rhn")
            nc.vector.reciprocal(rhn[:], hn_s[:])

            # ---- h^T (bf16) for the inner matmul ----
            psht = ptr.tile([128, H], F32, name="psht", tag="tr")
            for c in range(KC):
                nc.tensor.transpose(psht[:, ts(c, 128)], h_t[:, ts(c, 128)], ident[:])
            hT = hTpool.tile([128, KC, 128], BF16, name="hT")
            nc.vector.tensor_copy(hT[:], psht[:])

            # ---- inner = h @ M_unit^T ; e = exp(cos) ; s = sum(e) ----
            e_t = epool.tile([128, MEM], F32, name="e_t")
            s4 = stat.tile([128, NC4], F32, name="s4")
            for n in range(NC4):
                psi = pin.tile([128, 512], F32, name="psi")
                for c in range(KC):
                    nc.tensor.matmul(
                        psi[:], hT[:, c, :], MuT[:, c, ts(n, 512)],
                        start=(c == 0), stop=(c == KC - 1),
                    )
                nc.scalar.activation(
                    e_t[:, ts(n, 512)], psi[:], AF.Exp,
                    scale=rhn[:], accum_out=s4[:, n : n + 1],
                )
            s1 = stat.tile([128, 1], F32, name="s1")
            nc.vector.tensor_reduce(s1[:], s4[:], axis=AX, op=OP.add)
            rs = stat.tile([128, 1], F32, name="rs")
            nc.vector.reciprocal(rs[:], s1[:])

            # ---- w_r = e / s (in place), DMA out as new_prev_w_r ----
            nc.scalar.activation(e_t[:], e_t[:], AF.Copy, scale=rs[:])
            nc.sync.dma_start(nwr_d[ts(bt, 128), :], e_t[:])

            if stage < 3:
                continue
            # ---- read_i = w_r @ M ----
            psr = prd.tile([128, H], F32, name="psr")
            for mq in range(MC // 4):
                pst = ptr.tile([128, 512], F32, name="pst", tag="tr")
                for j in range(4):
                    m = 4 * mq + j
                    nc.tensor.transpose(
                        pst[:, ts(j, 128)], e_t[:, ts(m, 128)], ident[:]
                    )
                wrT = wrTpool.tile([128, 512], F32, name="wrT")
                nc.vector.tensor_copy(wrT[:], pst[:])
                for j in range(4):
                    m = 4 * mq + j
                    nc.tensor.matmul(
                        psr[:], wrT[:, ts(j, 128)], M_sb[:, m, :],
                        start=(m == 0), stop=(m == MC - 1),
                    )
            rd_t = rdpool.tile([128, H], F32, name="rd_t")
            nc.vector.tensor_copy(rd_t[:], psr[:])
            nc.sync.dma_start(out_d[ts(bt, 128), H : 2 * H], rd_t[:])

            if stage < 4:
                continue
            # ---- write head: one-hot of argmin(w_u), w_w ----
            wu_t = wupool.tile([128, MEM], F32, name="wu_t")
            nc.sync.dma_start(wu_t[:], wu_d[ts(bt, 128), :])
            pw_t = pwpool.tile([128, MEM], F32, name="pw_t")
            nc.sync.dma_start(pw_t[:], pw_d[ts(bt, 128), :])

            mn1 = stat.tile([128, 1], F32, name="mn1")
            nc.vector.tensor_reduce(mn1[:], wu_t[:], axis=AX, op=OP.min)
            oh_t = ohpool.tile([128, MEM], F32, name="oh_t", tag="ohshare")
            nc.vector.tensor_scalar(
                out=oh_t[:], in0=wu_t[:], scalar1=mn1[:],
                scalar2=ONE_MINUS_GATE, op0=OP.is_equal, op1=OP.mult,
            )
            # w_w = GATE*prev_w_r + oh  (in place on pw_t)
            nc.vector.scalar_tensor_tensor(
                out=pw_t[:], in0=pw_t[:], scalar=GATE, in1=oh_t[:],
                op0=OP.mult, op1=OP.add,
            )
            ww_tiles.append(pw_t)

            # ---- new_w_u = l2norm(GAMMA*w_u + w_r + w_w) (in place on wu_t) ----
            nc.vector.scalar_tensor_tensor(
                out=wu_t[:], in0=wu_t[:], scalar=GAMMA, in1=e_t[:],
                op0=OP.mult, op1=OP.add,
            )
            nc.vector.tensor_tensor(wu_t[:], wu_t[:], pw_t[:], op=OP.add)
            ssq = stat.tile([128, 1], F32, name="ssq")
            sq_scr = ohpool.tile([128, MEM], F32, name="sq_scr", tag="ohshare")
            nc.scalar.activation(sq_scr[:], wu_t[:], AF.Square, accum_out=ssq[:])
            ns_s = stat.tile([128, 1], F32, name="ns_s")
            nc.scalar.activation(ns_s[:], ssq[:], AF.Sqrt)
            rns = stat.tile([128, 1], F32, name="rns")
            nc.vector.reciprocal(rns[:], ns_s[:])
            nc.scalar.activation(wu_t[:], wu_t[:], AF.Copy, scale=rns[:])
            nc.sync.dma_start(nwu_d[ts(bt, 128), :], wu_t[:])

        if stage < 5:
            continue
        # ---- phase B: k_w partial accumulation over this group ----
        for m in range(MC):
            psk = pkw.tile([128, H], F32, name="psk")
            for t in range(GRP):
                nc.tensor.matmul(
                    psk[:], ww_tiles[t][:, ts(m, 128)], h_tiles[t][:],
                    start=(t == 0), stop=(t == GRP - 1),
                )
            if g == 0:
                nc.vector.tensor_copy(kw_acc[:, m, :], psk[:])
            else:
                nc.vector.tensor_tensor(
                    kw_acc[:, m, :], kw_acc[:, m, :], psk[:], op=OP.add
                )

    if stage < 5:
        return
    # ---------------- tail: ReduceScatter + new_M shard ----------------
    kwb_in = dram.tile([MEM, H], F32, name="kwb_in")
    kwb_out = dram.tile([MSH, H], F32, name="kwb_out")
    nc.sync.dma_start(
        kwb_in[:].rearrange("(m p) h -> p m h", p=128), kw_acc[:]
    )
    nc.gpsimd.collective_compute(
        "ReduceScatter",
        OP.add,
        replica_groups=[list(range(NCORES))],
        ins=[kwb_in[:]],
        outs=[kwb_out[:]],
    )
    for j in range(MSH // 128):
        ksh = rdpool.tile([128, H], F32, name="ksh", tag="rd_t")
        nc.sync.dma_start(ksh[:], kwb_out[ts(j, 128), :])
        msh = wrTpool.tile([128, H], F32, name="msh", tag="wrT")
        nc.sync.dma_start(msh[:], Msh_d[ts(j, 128), :])
        nc.vector.tensor_tensor(ksh[:], ksh[:], msh[:], op=OP.add)
        nsq = stat.tile([128, 1], F32, name="nsq")
        nscr = scrpool.tile([128, H], F32, name="nscr", tag="scr")
        nc.scalar.activation(
            nscr[:], ksh[:], AF.Square, accum_out=nsq[:]
        )
        nn_s = stat.tile([128, 1], F32, name="nn_s")
        nc.scalar.activation(nn_s[:], nsq[:], AF.Sqrt)
        nrn = stat.tile([128, 1], F32, name="nrn")
        nc.vector.reciprocal(nrn[:], nn_s[:])
        nc.scalar.activation(ksh[:], ksh[:], AF.Copy, scale=nrn[:])
        nc.sync.dma_start(newM_d[ts(j, 128), :], ksh[:])


def build_program(bl=BL, stage=5):
    nc = bacc.Bacc(
        "TRN2", target_bir_lowering=False, debug=False, num_devices=NCORES
    )
    nbt = bl // 128
    io = {}
    io["X_l"] = nc.dram_tensor("X_l", [bl, IN], F32, kind="ExternalInput").ap()
    io["W"] = nc.dram_tensor("W", [IN, H], F32, kind="ExternalInput").ap()
    io["b"] = nc.dram_tensor("b", [1, H], F32, kind="ExternalInput").ap()
    io["M"] = nc.dram_tensor("M", [MEM, H], F32, kind="ExternalInput").ap()
    io["M_shard"] = nc.dram_tensor(
        "M_shard", [MSH, H], F32, kind="ExternalInput"
    ).ap()
    io["w_u_l"] = nc.dram_tensor("w_u_l", [bl, MEM], F32, kind="ExternalInput").ap()
    io["prev_w_r_l"] = nc.dram_tensor(
        "prev_w_r_l", [bl, MEM], F32, kind="ExternalInput"
    ).ap()
    io["out_l"] = nc.dram_tensor(
        "out_l", [bl, 2 * H], F32, kind="ExternalOutput"
    ).ap()
    io["newM_l"] = nc.dram_tensor(
        "newM_l", [MSH, H], F32, kind="ExternalOutput"
    ).ap()
    io["new_w_u_l"] = nc.dram_tensor(
        "new_w_u_l", [bl, MEM], F32, kind="ExternalOutput"
    ).ap()
    io["new_prev_w_r_l"] = nc.dram_tensor(
        "new_prev_w_r_l", [bl, MEM], F32, kind="ExternalOutput"
    ).ap()

    from contextlib import ExitStack

    with tile.TileContext(nc) as tc:
        with ExitStack() as ctx:
            io["_ctx"] = ctx
            _build_body(nc, tc, io, nbt=nbt, stage=stage)
    nc.compile()
    return nc


_CACHED = {}


def _get_program():
    if "nc" not in _CACHED:
        _CACHED["nc"] = build_program()
    return _CACHED["nc"]


def _make_in_maps(X, W, b, M, w_u, prev_w_r, bl=BL):
    X = np.ascontiguousarray(X, dtype=np.float32)
    W = np.ascontiguousarray(W, dtype=np.float32)
    b = np.ascontiguousarray(b, dtype=np.float32).reshape(1, H)
    M = np.ascontiguousarray(M, dtype=np.float32)
    w_u = np.ascontiguousarray(w_u, dtype=np.float32)
    prev_w_r = np.ascontiguousarray(prev_w_r, dtype=np.float32)
    in_maps = []
    for c in range(NCORES):
        sl = slice(c * bl, (c + 1) * bl)
        in_maps.append(
            {
                "X_l": X[sl],
                "W": W,
                "b": b,
                "M": M,
                "M_shard": M[c * MSH : (c + 1) * MSH],
                "w_u_l": w_u[sl],
                "prev_w_r_l": prev_w_r[sl],
            }
        )
    return in_maps


def run_on_hw(X, W, b, M, w_u, prev_w_r, trace=False, **kw):
    nc = _get_program()
    in_maps = _make_in_maps(X, W, b, M, w_u, prev_w_r)
    res = run_bass_kernel_spmd(
        nc, in_maps, list(range(NCORES)), trace=trace, **kw
    )
    outs = res.results
    out = np.concatenate([outs[c]["out_l"] for c in range(NCORES)], axis=0)
    new_M = np.concatenate([outs[c]["newM_l"] for c in range(NCORES)], axis=0)
    new_w_u = np.concatenate(
        [outs[c]["new_w_u_l"] for c in range(NCORES)], axis=0
    )
    new_prev = np.concatenate(
        [outs[c]["new_prev_w_r_l"] for c in range(NCORES)], axis=0
    )
    return (out, new_M, new_w_u, new_prev), res


def kernel(X, W, b, M, w_u, prev_w_r):
    (out, new_M, new_w_u, new_prev), _ = run_on_hw(
        X, W, b, M, w_u, prev_w_r, trace=False
    )
    return out, new_M, new_w_u, new_prev
